# revision 1
# baseline (speedup 1.0000x reference)
"""Causal self-attention Trainium2 Bass kernel (v2).

Problem (hardcoded): B=4, S=2048, D=1024, H=16 heads, head_dim=64, fp32.
    qkv = x @ W_attn + b_attn; causal softmax attention; y @ W_proj + b_proj.

Sharding over 8 NeuronCores: core c -> (batch b = c//2, head-group g = c%2).
Each core computes, for its batch and its 8 heads (512 feature dims):
    Q^T, K^T [512f, 2048s] and V [2048s, 512f]  (fp32r, full-rate matmuls)
    flash-style causal attention in transposed layout:
        scores^T [128k, 512q] tiles = K^T.T @ Q^T   (per head, row-paired A/B)
        expS = exp(scores/8) via ACT (causal mask added in PSUM beforehand)
        out^T [64hd, 512q] += [V | ones].T @ expS   (denominator rides as row 64)
    normalization deferred + batched: denominators gathered per q-chunk into an
    [8, 512] tile (ACT copies), one reciprocal, K=1 PE broadcast, DVE multiply
    partial = y^T.T @ W_proj[group rows]  -> [2048, 1024]
Host: out[b] = partial(core 2b) + partial(core 2b+1) + b_proj + b_attn_v @ W_proj.
q/k biases are applied on-device (per-partition ACT bias); the v bias commutes
through softmax (rows sum to 1) so its projection is added on the host.
"""
import sys
if '/opt/trn_rl_repo' not in sys.path:
    sys.path.insert(0, '/opt/trn_rl_repo')

import numpy as np
import concourse.bass as bass
import concourse.mybir as mybir
import concourse.tile as tile
from concourse import bacc
from concourse import bass_utils

F32 = mybir.dt.float32
F32R = mybir.dt.float32r
AF = mybir.ActivationFunctionType
ALU = mybir.AluOpType

B, S, D, H, HD = 4, 2048, 1024, 16, 64
NCORES = 8
FPC = 512            # feature dims per core (8 heads * 64)
NPAIR = 4            # head pairs per core
DC = D // 128        # 8 contraction chunks for QKV/proj of x
NST = S // 128       # 16 s-tiles
MASKV = -30000.0     # exp(-30000/8) == 0 in fp32

_CACHE = {}


def _build_program():
    nc = bacc.Bacc("TRN2", target_bir_lowering=False, debug=False,
                   enable_asserts=False, num_devices=NCORES)

    xT_d = nc.dram_tensor("xT", [D, S], F32R, kind="ExternalInput").ap()
    wq_d = nc.dram_tensor("wq", [D, FPC], F32R, kind="ExternalInput").ap()
    wk_d = nc.dram_tensor("wk", [D, FPC], F32R, kind="ExternalInput").ap()
    wv_d = nc.dram_tensor("wv", [D, FPC], F32R, kind="ExternalInput").ap()
    wp_d = nc.dram_tensor("wp", [FPC, D], F32R, kind="ExternalInput").ap()
    bq_d = nc.dram_tensor("bq", [FPC], F32, kind="ExternalInput").ap()
    bk_d = nc.dram_tensor("bk", [FPC], F32, kind="ExternalInput").ap()
    out_d = nc.dram_tensor("out", [S, D], F32, kind="ExternalOutput").ap()

    from contextlib import ExitStack
    with tile.TileContext(nc) as tc, ExitStack() as ctx:
        persist = ctx.enter_context(tc.tile_pool(name="persist", bufs=1))
        QT = [persist.tile([128, S], F32R, name=f"qt{p}") for p in range(NPAIR)]
        KT = [persist.tile([128, S], F32R, name=f"kt{p}") for p in range(NPAIR)]
        # V tiles: [128 s, 8 heads, 65] -- col 64 is the ones column (denominator)
        Vt = [persist.tile([128, 8, 65], F32R, name=f"v{i}") for i in range(NST)]

        # ---------------- Phase 1: Q^T, K^T, V ----------------
        with ExitStack() as c1:
            wpool = c1.enter_context(tc.tile_pool(name="wpool", bufs=1))
            xpool = c1.enter_context(tc.tile_pool(name="xpool", bufs=2))
            p1ps = c1.enter_context(tc.tile_pool(name="p1ps", bufs=6, space="PSUM"))

            wq_sb = wpool.tile([128, DC, FPC], F32R, name="wq_sb")
            wk_sb = wpool.tile([128, DC, FPC], F32R, name="wk_sb")
            wv_sb = wpool.tile([128, DC, FPC], F32R, name="wv_sb")
            # per-chunk DMAs spread across queues (faster preamble)
            for c in range(DC):
                cs = slice(128 * c, 128 * c + 128)
                nc.sync.dma_start(wq_sb[:, c, :], wq_d[cs, :])
                nc.sync.dma_start(wk_sb[:, c, :], wk_d[cs, :])
                nc.sync.dma_start(wv_sb[:, c, :], wv_d[cs, :])
            bq_sb = wpool.tile([128, 4], F32, name="bq_sb")
            bk_sb = wpool.tile([128, 4], F32, name="bk_sb")
            nc.sync.dma_start(bq_sb[:], bq_d.rearrange("(c p) -> p c", p=128))
            nc.sync.dma_start(bk_sb[:], bk_d.rearrange("(c p) -> p c", p=128))

            onesv_f = wpool.tile([128, 8], F32, name="onesv_f")
            nc.gpsimd.memset(onesv_f[:], 1.0)
            for i in range(NST):
                nc.vector.tensor_copy(Vt[i][:, :, 64], onesv_f[:])

            for qtr in range(4):        # s-chunks of 512
                s0 = 512 * qtr
                xq = xpool.tile([128, DC, 512], F32R, name=f"xq{qtr}", tag="xq")
                for c in range(DC):
                    nc.sync.dma_start(xq[:, c, :],
                                      xT_d[128 * c:128 * c + 128, s0:s0 + 512])
                for f in range(4):      # feature chunks of 128 (= head pair)
                    for nm, w_sb, b_sb, dstT in (("q", wq_sb, bq_sb, QT),
                                                 ("k", wk_sb, bk_sb, KT)):
                        ps = p1ps.tile([128, 512], F32, name=f"ps{nm}{qtr}_{f}",
                                       tag="p1ps")
                        for c in range(DC):
                            nc.tensor.matmul(ps[:],
                                             w_sb[:, c, 128 * f:128 * f + 128],
                                             xq[:, c, :],
                                             start=(c == 0), stop=(c == DC - 1))
                        # psum->sbuf + per-feature bias on the idle ACT engine
                        nc.scalar.activation(dstT[f][:, s0:s0 + 512], ps[:],
                                             AF.Identity, bias=b_sb[:, f:f + 1])
                for ii in range(4):     # V s-tiles of 128 in this quarter
                    i = 4 * qtr + ii
                    psv = p1ps.tile([128, 512], F32, name=f"psv{i}", tag="p1ps")
                    for c in range(DC):
                        nc.tensor.matmul(psv[:],
                                         xq[:, c, 128 * ii:128 * ii + 128],
                                         wv_sb[:, c, :],
                                         start=(c == 0), stop=(c == DC - 1))
                    nc.vector.tensor_copy(
                        Vt[i][:, :, 0:64],
                        psv[:].rearrange("p (h u) -> p h u", h=8))

        # ---------------- Phase 2: attention + projection ----------------
        with ExitStack() as c2:
            per2 = c2.enter_context(tc.tile_pool(name="per2", bufs=1))
            expool = c2.enter_context(tc.tile_pool(name="expool", bufs=4))
            smpool = c2.enter_context(tc.tile_pool(name="smpool", bufs=2))
            outsb = c2.enter_context(tc.tile_pool(name="outsb", bufs=3))
            scps = c2.enter_context(tc.tile_pool(name="scps", bufs=2, space="PSUM"))
            pvps = c2.enter_context(tc.tile_pool(name="pvps", bufs=2, space="PSUM"))

            yT = [per2.tile([128, S], F32R, name=f"yt{p}") for p in range(NPAIR)]
            wp_sb = per2.tile([128, 4, D], F32R, name="wp_sb")
            for c in range(4):
                nc.sync.dma_start(wp_sb[:, c, :], wp_d[128 * c:128 * c + 128, :])

            # universal [128,128] strict-upper-triangle mask: 1.0 iff kp <= qf
            tri_f = per2.tile([128, 128], F32, name="tri_f")
            nc.gpsimd.memset(tri_f[:], 1.0)
            nc.gpsimd.affine_select(
                out=tri_f[:], in_=tri_f[:],
                compare_op=ALU.is_ge, fill=0.0,
                base=0, pattern=[[1, 128]], channel_multiplier=-1)
            tri = per2.tile([128, 128], F32R, name="tri")
            nc.vector.tensor_copy(tri[:], tri_f[:])

            ones64_f = per2.tile([1, 64], F32, name="ones64_f")
            nc.gpsimd.memset(ones64_f[:], 1.0)
            ones64 = per2.tile([1, 64], F32R, name="ones64")
            nc.vector.tensor_copy(ones64[:], ones64_f[:])

            for j in range(4):          # q-chunks of 512
                q0 = 512 * j
                nk = 4 * (j + 1)
                for p in range(NPAIR):
                    acc2 = pvps.tile([65, 1024], F32, name=f"acc{j}_{p}", tag="acc")

                    def emit_pv(t, ex, lo):
                        nc.tensor.matmul(acc2[:, lo:512], Vt[t][:, 2 * p, :],
                                         ex[:, lo:512],
                                         start=(t == 0), stop=(t == nk - 1))
                        nc.tensor.matmul(acc2[:, 512 + lo:1024],
                                         Vt[t][:, 2 * p + 1, :],
                                         ex[:, 512 + lo:1024],
                                         start=(t == 0), stop=(t == nk - 1))

                    pending = None
                    for t in range(nk):
                        k0 = 128 * t
                        sc = scps.tile([128, 1024], F32, name=f"sc{j}_{p}_{t}",
                                       tag="sc")
                        nc.tensor.matmul(sc[:, 0:512],
                                         KT[p][0:64, k0:k0 + 128],
                                         QT[p][0:64, q0:q0 + 512],
                                         start=True, stop=True)
                        nc.tensor.matmul(sc[:, 512:1024],
                                         KT[p][64:128, k0:k0 + 128],
                                         QT[p][64:128, q0:q0 + 512],
                                         start=True, stop=True)
                        ex = expool.tile([128, 1024], F32R, name=f"ex{j}_{p}_{t}",
                                         tag="ex")
                        oi = t - 4 * j
                        lo = max(0, 128 * oi)
                        nc.scalar.activation(ex[:, lo:1024], sc[:, lo:1024],
                                             AF.Exp, scale=0.125)
                        if oi >= 0:   # strict upper triangle of the diag block
                            nc.vector.tensor_tensor(
                                ex[:, lo:lo + 128], ex[:, lo:lo + 128],
                                tri[:], ALU.mult)
                            nc.vector.tensor_tensor(
                                ex[:, 512 + lo:512 + lo + 128],
                                ex[:, 512 + lo:512 + lo + 128],
                                tri[:], ALU.mult)
                        if pending is not None:
                            emit_pv(*pending)
                        pending = (t, ex, lo)
                    emit_pv(*pending)
                    # normalize: fast reciprocal of denom row, K=1 PE broadcast,
                    # then in-place multiply of the unnormalized yT copy
                    for hi in range(2):
                        acc = acc2[:, 512 * hi:512 * hi + 512]
                        r = 2 * p + hi
                        ys = yT[p][64 * hi:64 * hi + 64, q0:q0 + 512]
                        nc.vector.tensor_copy(ys, acc[0:64, :])
                        den = smpool.tile([1, 512], F32, name=f"den{j}_{r}",
                                          tag="den")
                        nc.scalar.copy(den[:], acc[64:65, :])
                        scr = smpool.tile([1, 512], F32, name=f"scr{j}_{r}",
                                          tag="scr")
                        rec = smpool.tile([1, 512], F32, name=f"rec{j}_{r}",
                                          tag="rec")
                        nc.vector.reciprocal_approx_accurate(
                            rec[:], den[:], scr[:])
                        rec_r = smpool.tile([1, 512], F32R, name=f"recr{j}_{r}",
                                            tag="rec_r")
                        nc.vector.tensor_copy(rec_r[:], rec[:])
                        bc = scps.tile([128, 1024], F32, name=f"bc{j}_{r}", tag="sc")
                        nc.tensor.matmul(bc[0:64, 0:512], ones64[:],
                                         rec_r[:], start=True, stop=True)
                        nc.vector.tensor_tensor(ys, bc[0:64, 0:512], ys, ALU.mult)
                # projection for this q-chunk (all pairs' yT just completed)
                def emit_proj(i, o):
                    po = scps.tile([128, 1024], F32, name=f"po{i}_{o}",
                                   tag="sc")
                    for p2 in range(NPAIR):
                        nc.tensor.matmul(po[:, 0:512],
                                         yT[p2][:, 128 * i:128 * i + 128],
                                         wp_sb[:, p2, 512 * o:512 * o + 512],
                                         start=(p2 == 0), stop=(p2 == 3))
                    ot = outsb.tile([128, 512], F32, name=f"ot{i}_{o}", tag="ot")
                    nc.vector.tensor_copy(ot[:], po[:, 0:512])
                    nc.sync.dma_start(
                        out_d[128 * i:128 * i + 128, 512 * o:512 * o + 512],
                        ot[:])
                for ii in range(4):
                    for o in range(2):
                        emit_proj(4 * j + ii, o)

    nc.compile()
    return nc


def _get_program():
    if "nc" not in _CACHE:
        _CACHE["nc"] = _build_program()
    return _CACHE["nc"]


def kernel(x, W_attn, b_attn, W_proj, b_proj, _trace=False, _trace_cores=None):
    x = np.asarray(x, np.float32)
    W_attn = np.asarray(W_attn, np.float32)
    b_attn = np.asarray(b_attn, np.float32)
    W_proj = np.asarray(W_proj, np.float32)
    b_proj = np.asarray(b_proj, np.float32)

    nc = _get_program()

    in_maps = []
    for c in range(NCORES):
        b, g = divmod(c, 2)
        gc = slice(FPC * g, FPC * g + FPC)
        in_maps.append({
            "xT": np.ascontiguousarray(x[b].T),
            "wq": np.ascontiguousarray(W_attn[:, 0 * D:1 * D][:, gc]),
            "wk": np.ascontiguousarray(W_attn[:, 1 * D:2 * D][:, gc]),
            "wv": np.ascontiguousarray(W_attn[:, 2 * D:3 * D][:, gc]),
            "wp": np.ascontiguousarray(W_proj[gc, :]),
            "bq": np.ascontiguousarray(b_attn[0 * D:1 * D][gc]),
            "bk": np.ascontiguousarray(b_attn[1 * D:2 * D][gc]),
        })

    kw = {}
    if _trace:
        kw = dict(trace=True, trace_cores=_trace_cores or [0])
    res = bass_utils.run_bass_kernel_spmd(nc, in_maps, core_ids=list(range(NCORES)),
                                          **kw)

    # host-side reduction: v-bias commutes through softmax -> fold via W_proj
    corr = b_proj + b_attn[2 * D:3 * D] @ W_proj
    out = np.empty((B, S, D), np.float32)
    for b in range(B):
        out[b] = res.results[2 * b]["out"] + res.results[2 * b + 1]["out"] + corr

    if _trace:
        kernel._last_results = res
    return out



# revision 4
# speedup vs baseline: 1.5651x; 1.5651x over previous
"""Causal self-attention Trainium2 Bass kernel (v3, bf16).

Problem (hardcoded): B=4, S=2048, D=1024, H=16 heads, head_dim=64.
    qkv = x @ W_attn + b_attn; causal softmax attention; y @ W_proj + b_proj.

Sharding over 8 NeuronCores: core c -> (batch b = c//2, head-group g = c%2).
Each core computes, for its batch and its 8 heads (512 feature dims):
    Q^T, K^T [512f, 2048s] and V [2048s, 512f] in bf16
    flash-style causal attention in transposed layout, per head:
        scores^T [128k, 512q] = K^T.T @ Q^T  (two heads concurrent via PE
        row groups 0/64), exp on ACT (bf16 out), causal mask for diagonal
        blocks via gpsimd affine_select, PV accumulation [65hd, 512q] with a
        ones column carrying the softmax denominator.
    normalization: DVE reciprocal of the denominator row, gpsimd
    partition_broadcast, DVE multiply writing bf16 y^T.
    projection: y^T.T @ W_proj -> [2048, 1024] fp32 partial.
All matmuls bf16 (separate LDWEIGHTS with FWL overlaps the previous matmul;
fp32r would self-load weights at ~180ns serialized per matmul).
QKV and projection matmuls are interleaved into the attention stream at
sub-tile granularity so the PE never idles while ACT exp catches up.
Host: out[b] = partial(core 2b) + partial(core 2b+1) + b_proj + b_attn_v @ W_proj.
"""
import sys
if '/opt/trn_rl_repo' not in sys.path:
    sys.path.insert(0, '/opt/trn_rl_repo')

import numpy as np
import ml_dtypes
import concourse.bass as bass
import concourse.mybir as mybir
import concourse.tile as tile
from concourse import bacc
from concourse import bass_utils
from concourse import library_config

F32 = mybir.dt.float32
BF16 = mybir.dt.bfloat16
AF = mybir.ActivationFunctionType
ALU = mybir.AluOpType

B, S, D, H, HD = 4, 2048, 1024, 16, 64
NCORES = 8
FPC = 512            # feature dims per core (8 heads * 64)
NPAIR = 4            # head pairs per core
DC = D // 128        # 8 contraction chunks
NST = S // 128       # 16 s-tiles

_CACHE = {}


def _build_program():
    nc = bacc.Bacc("TRN2", target_bir_lowering=False, debug=False,
                   enable_asserts=False, num_devices=NCORES)

    xT_d = nc.dram_tensor("xT", [D, S], BF16, kind="ExternalInput").ap()
    wq_d = nc.dram_tensor("wq", [D, FPC], BF16, kind="ExternalInput").ap()
    wk_d = nc.dram_tensor("wk", [D, FPC], BF16, kind="ExternalInput").ap()
    wv_d = nc.dram_tensor("wv", [D, FPC], BF16, kind="ExternalInput").ap()
    wp_d = nc.dram_tensor("wp", [FPC, D], BF16, kind="ExternalInput").ap()
    bq_d = nc.dram_tensor("bq", [FPC], F32, kind="ExternalInput").ap()
    bk_d = nc.dram_tensor("bk", [FPC], F32, kind="ExternalInput").ap()
    out_d = nc.dram_tensor("out", [S, D], F32, kind="ExternalOutput").ap()

    from contextlib import ExitStack
    with tile.TileContext(nc) as tc, ExitStack() as ctx:
        persist = ctx.enter_context(tc.tile_pool(name="persist", bufs=1))
        xpool = ctx.enter_context(tc.tile_pool(name="xpool", bufs=2))
        expool = ctx.enter_context(tc.tile_pool(name="expool", bufs=8))
        smpool = ctx.enter_context(tc.tile_pool(name="smpool", bufs=3))
        outsb = ctx.enter_context(tc.tile_pool(name="outsb", bufs=3))
        scps = ctx.enter_context(tc.tile_pool(name="scps", bufs=3, space="PSUM"))
        wps = ctx.enter_context(tc.tile_pool(name="wps", bufs=2, space="PSUM"))
        accps = ctx.enter_context(tc.tile_pool(name="accps", bufs=3, space="PSUM"))

        nc.gpsimd.load_library(library_config.attn)

        QT = [persist.tile([128, S], BF16, name=f"qt{p}") for p in range(NPAIR)]
        KT = [persist.tile([128, S], BF16, name=f"kt{p}") for p in range(NPAIR)]
        yT = [persist.tile([128, S], BF16, name=f"yt{p}") for p in range(NPAIR)]
        # V tiles: [128 s, 8 heads, 65] -- col 64 is the ones column (denominator)
        Vt = [persist.tile([128, 8, 65], BF16, name=f"v{i}") for i in range(NST)]

        wq_sb = persist.tile([128, DC, FPC], BF16, name="wq_sb")
        wk_sb = persist.tile([128, DC, FPC], BF16, name="wk_sb")
        wv_sb = persist.tile([128, DC, FPC], BF16, name="wv_sb")
        wp_sb = persist.tile([128, 4, D], BF16, name="wp_sb")
        for c in range(DC):
            cs = slice(128 * c, 128 * c + 128)
            nc.sync.dma_start(wq_sb[:, c, :], wq_d[cs, :])
            nc.sync.dma_start(wk_sb[:, c, :], wk_d[cs, :])
            nc.sync.dma_start(wv_sb[:, c, :], wv_d[cs, :])
        for c in range(4):
            nc.sync.dma_start(wp_sb[:, c, :], wp_d[128 * c:128 * c + 128, :])
        bq_sb = persist.tile([128, 4], F32, name="bq_sb")
        bk_sb = persist.tile([128, 4], F32, name="bk_sb")
        nc.sync.dma_start(bq_sb[:], bq_d.rearrange("(c p) -> p c", p=128))
        nc.sync.dma_start(bk_sb[:], bk_d.rearrange("(c p) -> p c", p=128))

        onesv = persist.tile([128, 8], BF16, name="onesv")
        nc.gpsimd.memset(onesv[:], 1.0)
        for i in range(NST):
            nc.vector.tensor_copy(Vt[i][:, :, 64], onesv[:])

        # ---- emission helpers ------------------------------------------
        def qk_units(seg, p, xq):
            """4 closures: Q(p) first/second half, K(p) first/second half."""
            s0 = 512 * seg
            st = {}

            def mk(nm, w_sb, b_sb, dstT):
                def u0():
                    ps = wps.tile([128, 512], F32, tag="wps",
                                  name=f"ps{nm}{seg}_{p}")
                    for c in range(4):
                        nc.tensor.matmul(ps[:], w_sb[:, c, 128 * p:128 * p + 128],
                                         xq[:, c, :], start=(c == 0), stop=False)
                    st[nm] = ps

                def u1():
                    ps = st[nm]
                    for c in range(4, DC):
                        nc.tensor.matmul(ps[:], w_sb[:, c, 128 * p:128 * p + 128],
                                         xq[:, c, :], start=False,
                                         stop=(c == DC - 1))
                    nc.vector.tensor_scalar_add(dstT[p][:, s0:s0 + 512], ps[:],
                                                b_sb[:, p:p + 1])
                return [u0, u1]

            return mk("q", wq_sb, bq_sb, QT) + mk("k", wk_sb, bk_sb, KT)

        def v_units(seg, xq):
            """4 closures, one V s-tile each."""
            us = []
            for ii in range(4):
                i = 4 * seg + ii

                def u(i=i, ii=ii):
                    ps = wps.tile([128, 512], F32, tag="wps", name=f"psv{i}")
                    for c in range(DC):
                        nc.tensor.matmul(ps[:], xq[:, c, 128 * ii:128 * ii + 128],
                                         wv_sb[:, c, :], start=(c == 0),
                                         stop=(c == DC - 1))
                    nc.vector.tensor_copy(
                        Vt[i][:, :, 0:64],
                        ps[:].rearrange("p (h u) -> p h u", h=8))
                us.append(u)
            return us

        def proj_units(j):
            """8 closures, one [128s, 512d] output tile each."""
            us = []
            for i4 in range(4):
                for o in range(2):
                    i = 4 * j + i4

                    def u(i=i, o=o):
                        po = wps.tile([128, 512], F32, tag="wps",
                                      name=f"po{i}_{o}")
                        for p2 in range(NPAIR):
                            nc.tensor.matmul(po[:],
                                             yT[p2][:, 128 * i:128 * i + 128],
                                             wp_sb[:, p2, 512 * o:512 * o + 512],
                                             start=(p2 == 0), stop=(p2 == 3))
                        ot = outsb.tile([128, 512], F32, tag="ot",
                                        name=f"ot{i}_{o}")
                        nc.vector.tensor_copy(ot[:], po[:])
                        nc.sync.dma_start(
                            out_d[128 * i:128 * i + 128, 512 * o:512 * o + 512],
                            ot[:])
                    us.append(u)
            return us

        def att_pair(j, p, inject):
            q0 = 512 * j
            nk = 4 * (j + 1)
            accA = accps.tile([65, 512], F32, tag="acc", name=f"accA{j}_{p}")
            accB = accps.tile([65, 512], F32, tag="acc", name=f"accB{j}_{p}")

            def emit_pv(t, exA, exB, lo):
                nc.tensor.matmul(accA[:, lo:512], Vt[t][:, 2 * p, :],
                                 exA[:, lo:512], start=(t == 0),
                                 stop=(t == nk - 1))
                nc.tensor.matmul(accB[:, lo:512], Vt[t][:, 2 * p + 1, :],
                                 exB[:, lo:512], start=(t == 0),
                                 stop=(t == nk - 1))

            pending = None
            for t in range(nk):
                k0 = 128 * t
                scA = scps.tile([128, 512], F32, tag="sc", name=f"scA{j}_{p}_{t}")
                scB = scps.tile([128, 512], F32, tag="sc", name=f"scB{j}_{p}_{t}")
                nc.tensor.matmul(scA[:], KT[p][0:64, k0:k0 + 128],
                                 QT[p][0:64, q0:q0 + 512], start=True, stop=True)
                nc.tensor.matmul(scB[:], KT[p][64:128, k0:k0 + 128],
                                 QT[p][64:128, q0:q0 + 512], start=True, stop=True)
                oi = t - 4 * j
                lo = max(0, 128 * oi)
                exA = expool.tile([128, 512], BF16, tag="ex",
                                  name=f"exA{j}_{p}_{t}")
                exB = expool.tile([128, 512], BF16, tag="ex",
                                  name=f"exB{j}_{p}_{t}")
                nc.scalar.activation(exA[:, lo:512], scA[:, lo:512], AF.Exp,
                                     scale=0.125)
                nc.scalar.activation(exB[:, lo:512], scB[:, lo:512], AF.Exp,
                                     scale=0.125)
                if oi >= 0:   # strict upper triangle of the diagonal block
                    for exh in (exA, exB):
                        nc.gpsimd.affine_select(
                            out=exh[:, lo:lo + 128], in_=exh[:, lo:lo + 128],
                            compare_op=ALU.is_ge, fill=0.0,
                            base=0, pattern=[[1, 128]], channel_multiplier=-1)
                if pending is not None:
                    emit_pv(*pending)
                pending = (t, exA, exB, lo)
                inject()
            emit_pv(*pending)

            # normalization (no PE involvement)
            denA = smpool.tile([1, 512], F32, tag="den", name=f"denA{j}_{p}")
            denB = smpool.tile([1, 512], F32, tag="den", name=f"denB{j}_{p}")
            nc.vector.tensor_copy(denA[:], accA[64:65, :])
            nc.vector.tensor_copy(denB[:], accB[64:65, :])
            recA = smpool.tile([1, 512], F32, tag="rec", name=f"recA{j}_{p}")
            recB = smpool.tile([1, 512], F32, tag="rec", name=f"recB{j}_{p}")
            nc.vector.reciprocal_approx_fast(recA[:], denA[:])
            nc.vector.reciprocal_approx_fast(recB[:], denB[:])
            bcA = smpool.tile([64, 512], F32, tag="bc", name=f"bcA{j}_{p}")
            bcB = smpool.tile([64, 512], F32, tag="bc", name=f"bcB{j}_{p}")
            nc.gpsimd.partition_broadcast(bcA[:], recA[0:1, :], channels=64)
            nc.gpsimd.partition_broadcast(bcB[:], recB[0:1, :], channels=64)
            nc.vector.tensor_tensor(yT[p][0:64, q0:q0 + 512], accA[0:64, :],
                                    bcA[:], ALU.mult)
            nc.vector.tensor_tensor(yT[p][64:128, q0:q0 + 512], accB[0:64, :],
                                    bcB[:], ALU.mult)

        # ---- main schedule ---------------------------------------------
        for seg in range(4):
            s0 = 512 * seg
            xq = xpool.tile([128, DC, 512], BF16, name=f"xq{seg}", tag="xq")
            for c in range(DC):
                nc.sync.dma_start(xq[:, c, :],
                                  xT_d[128 * c:128 * c + 128, s0:s0 + 512])
            # pair-0 QK and all V tiles up front
            for u in qk_units(seg, 0, xq):
                u()
            for u in v_units(seg, xq):
                u()
            # remaining QK + previous chunk's projection injected into the
            # attention stream
            queues = [[] for _ in range(NPAIR)]
            for pp in (1, 2, 3):
                queues[pp - 1] += qk_units(seg, pp, xq)
            if seg >= 1:
                pu = proj_units(seg - 1)
                for p in range(NPAIR):
                    queues[p] += pu[2 * p:2 * p + 2]
            for p in range(NPAIR):
                q = queues[p]

                def inject(q=q):
                    if q:
                        q.pop(0)()
                att_pair(seg, p, inject)
                while q:   # flush any leftovers at pair end
                    q.pop(0)()
        for u in proj_units(3):
            u()

    nc.compile()
    return nc


def _get_program():
    if "nc" not in _CACHE:
        _CACHE["nc"] = _build_program()
    return _CACHE["nc"]


def kernel(x, W_attn, b_attn, W_proj, b_proj, _trace=False, _trace_cores=None):
    x = np.asarray(x, np.float32)
    W_attn = np.asarray(W_attn, np.float32)
    b_attn = np.asarray(b_attn, np.float32)
    W_proj = np.asarray(W_proj, np.float32)
    b_proj = np.asarray(b_proj, np.float32)

    nc = _get_program()

    bf16 = ml_dtypes.bfloat16
    x16 = x.astype(bf16)
    Wa16 = W_attn.astype(bf16)
    Wp16 = W_proj.astype(bf16)

    in_maps = []
    for c in range(NCORES):
        b, g = divmod(c, 2)
        gc = slice(FPC * g, FPC * g + FPC)
        in_maps.append({
            "xT": np.ascontiguousarray(x16[b].T),
            "wq": np.ascontiguousarray(Wa16[:, 0 * D:1 * D][:, gc]),
            "wk": np.ascontiguousarray(Wa16[:, 1 * D:2 * D][:, gc]),
            "wv": np.ascontiguousarray(Wa16[:, 2 * D:3 * D][:, gc]),
            "wp": np.ascontiguousarray(Wp16[gc, :]),
            "bq": np.ascontiguousarray(b_attn[0 * D:1 * D][gc]),
            "bk": np.ascontiguousarray(b_attn[1 * D:2 * D][gc]),
        })

    kw = {}
    if _trace:
        kw = dict(trace=True, trace_cores=_trace_cores or [0])
    res = bass_utils.run_bass_kernel_spmd(nc, in_maps, core_ids=list(range(NCORES)),
                                          **kw)

    # host-side reduction: v-bias commutes through softmax -> fold via W_proj
    corr = b_proj + b_attn[2 * D:3 * D] @ W_proj
    out = np.empty((B, S, D), np.float32)
    for b in range(B):
        out[b] = res.results[2 * b]["out"] + res.results[2 * b + 1]["out"] + corr

    if _trace:
        kernel._last_results = res
    return out


# revision 17
# speedup vs baseline: 1.7055x; 1.0897x over previous
"""Causal self-attention Trainium2 Bass kernel (v3, bf16).

Problem (hardcoded): B=4, S=2048, D=1024, H=16 heads, head_dim=64.
    qkv = x @ W_attn + b_attn; causal softmax attention; y @ W_proj + b_proj.

Sharding over 8 NeuronCores: core c -> (batch b = c//2, head-group g = c%2).
Each core computes, for its batch and its 8 heads (512 feature dims):
    Q^T, K^T [512f, 2048s] and V [2048s, 512f] in bf16
    flash-style causal attention in transposed layout, per head:
        scores^T [128k, 512q] = K^T.T @ Q^T  (two heads concurrent via PE
        row groups 0/64), exp on ACT (bf16 out), causal mask for diagonal
        blocks via gpsimd affine_select, PV accumulation [65hd, 512q] with a
        ones column carrying the softmax denominator.
    normalization: DVE reciprocal of the denominator row, gpsimd
    partition_broadcast, DVE multiply writing bf16 y^T.
    projection: y^T.T @ W_proj -> [2048, 1024] fp32 partial.
All matmuls bf16 (separate LDWEIGHTS with FWL overlaps the previous matmul;
fp32r would self-load weights at ~180ns serialized per matmul).
QKV and projection matmuls are interleaved into the attention stream at
sub-tile granularity so the PE never idles while ACT exp catches up.
Host: out[b] = partial(core 2b) + partial(core 2b+1) + b_proj + b_attn_v @ W_proj.
"""
import sys
if '/opt/trn_rl_repo' not in sys.path:
    sys.path.insert(0, '/opt/trn_rl_repo')

import numpy as np
import ml_dtypes
import concourse.bass as bass
import concourse.mybir as mybir
import concourse.tile as tile
from concourse import bacc
from concourse import bass_utils
from concourse import library_config

F32 = mybir.dt.float32
BF16 = mybir.dt.bfloat16
AF = mybir.ActivationFunctionType
ALU = mybir.AluOpType

B, S, D, H, HD = 4, 2048, 1024, 16, 64
NCORES = 8
FPC = 512            # feature dims per core (8 heads * 64)
NPAIR = 4            # head pairs per core
DC = D // 128        # 8 contraction chunks
NST = S // 128       # 16 s-tiles

_CACHE = {}


def _build_program():
    nc = bacc.Bacc("TRN2", target_bir_lowering=False, debug=False,
                   enable_asserts=False, num_devices=NCORES)

    xT_d = nc.dram_tensor("xT", [D, S], BF16, kind="ExternalInput").ap()
    wq_d = nc.dram_tensor("wq", [D, FPC], BF16, kind="ExternalInput").ap()
    wk_d = nc.dram_tensor("wk", [D, FPC], BF16, kind="ExternalInput").ap()
    wv_d = nc.dram_tensor("wv", [D, FPC], BF16, kind="ExternalInput").ap()
    wp_d = nc.dram_tensor("wp", [FPC, D], BF16, kind="ExternalInput").ap()
    bq_d = nc.dram_tensor("bq", [FPC], F32, kind="ExternalInput").ap()
    bk_d = nc.dram_tensor("bk", [FPC], F32, kind="ExternalInput").ap()
    out_d = nc.dram_tensor("out", [S, D], F32, kind="ExternalOutput").ap()

    from contextlib import ExitStack
    with tile.TileContext(nc) as tc, ExitStack() as ctx:
        persist = ctx.enter_context(tc.tile_pool(name="persist", bufs=1))
        xpool = ctx.enter_context(tc.tile_pool(name="xpool", bufs=2))
        expool = ctx.enter_context(tc.tile_pool(name="expool", bufs=4))
        smpool = ctx.enter_context(tc.tile_pool(name="smpool", bufs=3))
        outsb = ctx.enter_context(tc.tile_pool(name="outsb", bufs=3))
        scps = ctx.enter_context(tc.tile_pool(name="scps", bufs=2, space="PSUM"))
        wps = ctx.enter_context(tc.tile_pool(name="wps", bufs=2, space="PSUM"))
        accps = ctx.enter_context(tc.tile_pool(name="accps", bufs=2, space="PSUM"))

        nc.gpsimd.load_library(library_config.attn)

        QT = [persist.tile([128, S], BF16, name=f"qt{p}") for p in range(NPAIR)]
        KT = [persist.tile([128, S], BF16, name=f"kt{p}") for p in range(NPAIR)]
        yT = [persist.tile([128, S], BF16, name=f"yt{p}") for p in range(NPAIR)]
        # V tiles: [128 s, 8 heads, 65] -- col 64 is the ones column (denominator)
        Vt = [persist.tile([128, 8, 65], BF16, name=f"v{i}") for i in range(NST)]

        # x chunk for segment 0 issued before the weights so the first
        # matmul isn't stuck behind serialized DMA descriptor issue
        xq0 = xpool.tile([128, DC, 512], BF16, name="xq_seg0", tag="xq")
        nc.sync.dma_start(xq0[:], xT_d[:, 0:512].rearrange("(c p) s -> p c s",
                                                           p=128))
        wq_sb = persist.tile([128, DC, FPC], BF16, name="wq_sb")
        wk_sb = persist.tile([128, DC, FPC], BF16, name="wk_sb")
        wv_sb = persist.tile([128, DC, FPC], BF16, name="wv_sb")
        wp_sb = persist.tile([128, 4, D], BF16, name="wp_sb")
        nc.sync.dma_start(wq_sb[:], wq_d.rearrange("(c p) f -> p c f", p=128))
        nc.sync.dma_start(wk_sb[:], wk_d.rearrange("(c p) f -> p c f", p=128))
        nc.sync.dma_start(wv_sb[:], wv_d.rearrange("(c p) f -> p c f", p=128))
        nc.sync.dma_start(wp_sb[:], wp_d.rearrange("(c p) f -> p c f", p=128))
        bq_sb = persist.tile([128, 4], F32, name="bq_sb")
        bk_sb = persist.tile([128, 4], F32, name="bk_sb")
        nc.sync.dma_start(bq_sb[:], bq_d.rearrange("(c p) -> p c", p=128))
        nc.sync.dma_start(bk_sb[:], bk_d.rearrange("(c p) -> p c", p=128))

        onesv = persist.tile([128, 8], BF16, name="onesv")
        nc.gpsimd.memset(onesv[:], 1.0)
        for i in range(NST):
            nc.vector.tensor_copy(Vt[i][:, :, 64], onesv[:])

        # ---- emission helpers ------------------------------------------
        def qk_units(seg, p, xq):
            """4 closures: Q(p) first/second half, K(p) first/second half."""
            s0 = 512 * seg
            st = {}

            def mk(nm, w_sb, b_sb, dstT):
                def u0():
                    ps = wps.tile([128, 512], F32, tag="wps",
                                  name=f"ps{nm}{seg}_{p}")
                    for c in range(4):
                        nc.tensor.matmul(ps[:], w_sb[:, c, 128 * p:128 * p + 128],
                                         xq[:, c, :], start=(c == 0), stop=False)
                    st[nm] = ps

                def u1():
                    ps = st[nm]
                    for c in range(4, DC):
                        nc.tensor.matmul(ps[:], w_sb[:, c, 128 * p:128 * p + 128],
                                         xq[:, c, :], start=False,
                                         stop=(c == DC - 1))
                    nc.vector.tensor_scalar_add(dstT[p][:, s0:s0 + 512], ps[:],
                                                b_sb[:, p:p + 1])
                return [u0, u1]

            return mk("q", wq_sb, bq_sb, QT) + mk("k", wk_sb, bk_sb, KT)

        def v_units(seg, xq):
            """4 closures, one V s-tile each."""
            us = []
            for ii in range(4):
                i = 4 * seg + ii

                def u(i=i, ii=ii):
                    ps = wps.tile([128, 512], F32, tag="wps", name=f"psv{i}")
                    for c in range(DC):
                        nc.tensor.matmul(ps[:], xq[:, c, 128 * ii:128 * ii + 128],
                                         wv_sb[:, c, :], start=(c == 0),
                                         stop=(c == DC - 1))
                    nc.vector.tensor_copy(
                        Vt[i][:, :, 0:64],
                        ps[:].rearrange("p (h u) -> p h u", h=8))
                us.append(u)
            return us

        def proj_units(j):
            """8 closures, one [128s, 512d] output tile each."""
            us = []
            for i4 in range(4):
                for o in range(2):
                    i = 4 * j + i4

                    def u(i=i, o=o):
                        po = wps.tile([128, 512], F32, tag="wps",
                                      name=f"po{i}_{o}")
                        for p2 in range(NPAIR):
                            nc.tensor.matmul(po[:],
                                             yT[p2][:, 128 * i:128 * i + 128],
                                             wp_sb[:, p2, 512 * o:512 * o + 512],
                                             start=(p2 == 0), stop=(p2 == 3))
                        ot = outsb.tile([128, 512], F32, tag="ot",
                                        name=f"ot{i}_{o}")
                        nc.vector.tensor_copy(ot[:], po[:])
                        nc.sync.dma_start(
                            out_d[128 * i:128 * i + 128, 512 * o:512 * o + 512],
                            ot[:])
                    us.append(u)
            return us

        def att_pair(j, p, inject):
            q0 = 512 * j
            nk = 4 * (j + 1)
            accA = accps.tile([65, 512], F32, tag="acc", name=f"accA{j}_{p}")
            accB = accps.tile([65, 512], F32, tag="acc", name=f"accB{j}_{p}")

            def emit_pv(t, ex, lo):
                nc.tensor.matmul(accA[:, lo:512], Vt[t][:, 2 * p, :],
                                 ex[:, lo:512], start=(t == 0),
                                 stop=(t == nk - 1))
                nc.tensor.matmul(accB[:, lo:512], Vt[t][:, 2 * p + 1, :],
                                 ex[:, 512 + lo:1024], start=(t == 0),
                                 stop=(t == nk - 1))

            pending = None
            for t in range(nk):
                k0 = 128 * t
                oi = t - 4 * j
                lo = max(0, 128 * oi)
                sc = scps.tile([128, 1024], F32, tag="sc", name=f"sc{j}_{p}_{t}")
                nc.tensor.matmul(sc[:, lo:512], KT[p][0:64, k0:k0 + 128],
                                 QT[p][0:64, q0 + lo:q0 + 512],
                                 start=True, stop=True)
                nc.tensor.matmul(sc[:, 512 + lo:1024], KT[p][64:128, k0:k0 + 128],
                                 QT[p][64:128, q0 + lo:q0 + 512],
                                 start=True, stop=True)
                ex = expool.tile([128, 1024], BF16, tag="ex",
                                 name=f"ex{j}_{p}_{t}")
                if oi >= 0:   # diagonal: skip head B's dead zone
                    nc.scalar.activation(ex[:, lo:512], sc[:, lo:512], AF.Exp,
                                         scale=0.125)
                    nc.scalar.activation(ex[:, 512 + lo:1024],
                                         sc[:, 512 + lo:1024], AF.Exp,
                                         scale=0.125)
                    # strict upper triangle of the diagonal block
                    for lo2 in (lo, 512 + lo):
                        nc.gpsimd.affine_select(
                            out=ex[:, lo2:lo2 + 128], in_=ex[:, lo2:lo2 + 128],
                            compare_op=ALU.is_ge, fill=0.0,
                            base=0, pattern=[[1, 128]], channel_multiplier=-1)
                else:
                    nc.scalar.activation(ex[:], sc[:], AF.Exp, scale=0.125)
                if pending is not None:
                    emit_pv(*pending)
                pending = (t, ex, lo)
                inject()
            emit_pv(*pending)

            # normalization: evacuate PSUM fast (recip + unnormalized copy),
            # then broadcast the reciprocal and scale yT in place — the PE
            # and the acc slots never wait on the broadcast
            denA = smpool.tile([1, 512], F32, tag="denA", name=f"denA{j}_{p}")
            denB = smpool.tile([1, 512], F32, tag="denB", name=f"denB{j}_{p}")
            recA = smpool.tile([1, 512], F32, tag="rec", name=f"recA{j}_{p}")
            recB = smpool.tile([1, 512], F32, tag="rec", name=f"recB{j}_{p}")
            bcA = smpool.tile([128, 512], F32, tag="bcA", name=f"bcA{j}_{p}")
            bcB = smpool.tile([128, 512], F32, tag="bcB", name=f"bcB{j}_{p}")
            for acc, den, rec, bc, hi in ((accA, denA, recA, bcA, 0),
                                          (accB, denB, recB, bcB, 1)):
                ys = yT[p][64 * hi:64 * hi + 64, q0:q0 + 512]
                nc.vector.tensor_copy(den[:], acc[64:65, :])
                nc.vector.reciprocal_approx_fast(rec[:], den[:])
                nc.vector.tensor_copy(ys, acc[0:64, :])
                nc.gpsimd.partition_broadcast(bc[:], rec[0:1, :], channels=128)
            for bc, hi in ((bcA, 0), (bcB, 1)):
                ys = yT[p][64 * hi:64 * hi + 64, q0:q0 + 512]
                nc.vector.tensor_tensor(ys, ys,
                                        bc[64 * hi:64 * hi + 64, :], ALU.mult)

        # ---- main schedule ---------------------------------------------
        for seg in range(4):
            s0 = 512 * seg
            if seg == 0:
                xq = xq0
            else:
                xq = xpool.tile([128, DC, 512], BF16, name=f"xq{seg}", tag="xq")
                nc.sync.dma_start(
                    xq[:], xT_d[:, s0:s0 + 512].rearrange("(c p) s -> p c s",
                                                          p=128))
            # pair-0 QK and all V tiles up front
            for u in qk_units(seg, 0, xq):
                u()
            for u in v_units(seg, xq):
                u()
            # remaining QK + previous chunk's projection injected into the
            # attention stream
            queues = [[] for _ in range(NPAIR)]
            for pp in (1, 2, 3):
                queues[pp - 1] += qk_units(seg, pp, xq)
            if seg >= 1:
                pu = proj_units(seg - 1)
                for p in range(NPAIR):
                    queues[p] += pu[2 * p:2 * p + 2]
            for p in range(NPAIR):
                q = queues[p]

                def inject(q=q):
                    if q:
                        q.pop(0)()
                att_pair(seg, p, inject)
                while q:   # flush any leftovers at pair end
                    q.pop(0)()
        for u in proj_units(3):
            u()

    nc.compile()
    return nc


def _get_program():
    if "nc" not in _CACHE:
        _CACHE["nc"] = _build_program()
    return _CACHE["nc"]


def kernel(x, W_attn, b_attn, W_proj, b_proj, _trace=False, _trace_cores=None):
    x = np.asarray(x, np.float32)
    W_attn = np.asarray(W_attn, np.float32)
    b_attn = np.asarray(b_attn, np.float32)
    W_proj = np.asarray(W_proj, np.float32)
    b_proj = np.asarray(b_proj, np.float32)

    nc = _get_program()

    bf16 = ml_dtypes.bfloat16
    x16 = x.astype(bf16)
    Wa16 = W_attn.astype(bf16)
    Wp16 = W_proj.astype(bf16)

    in_maps = []
    for c in range(NCORES):
        b, g = divmod(c, 2)
        gc = slice(FPC * g, FPC * g + FPC)
        in_maps.append({
            "xT": np.ascontiguousarray(x16[b].T),
            "wq": np.ascontiguousarray(Wa16[:, 0 * D:1 * D][:, gc]),
            "wk": np.ascontiguousarray(Wa16[:, 1 * D:2 * D][:, gc]),
            "wv": np.ascontiguousarray(Wa16[:, 2 * D:3 * D][:, gc]),
            "wp": np.ascontiguousarray(Wp16[gc, :]),
            "bq": np.ascontiguousarray(b_attn[0 * D:1 * D][gc]),
            "bk": np.ascontiguousarray(b_attn[1 * D:2 * D][gc]),
        })

    kw = {}
    if _trace:
        kw = dict(trace=True, trace_cores=_trace_cores or [0])
    res = bass_utils.run_bass_kernel_spmd(nc, in_maps, core_ids=list(range(NCORES)),
                                          **kw)

    # host-side reduction: v-bias commutes through softmax -> fold via W_proj
    corr = b_proj + b_attn[2 * D:3 * D] @ W_proj
    out = np.empty((B, S, D), np.float32)
    for b in range(B):
        out[b] = res.results[2 * b]["out"] + res.results[2 * b + 1]["out"] + corr

    if _trace:
        kernel._last_results = res
    return out
